# revision 1
# baseline (speedup 1.0000x reference)
"""DistMult decoder on 8 Trainium2 NeuronCores.

reference: out[k, i, j] = sigmoid( sum_d x_i[i, d] * relations[k, d] * x_j[j, d] )
shapes: x_i [4096, 128] f32, x_j [4096, 128] f32, relations [8, 128] f32
output: [8, 4096, 4096] f32 (512 MiB)

Sharding: rows of x_i (N_i axis) split across the 8 cores (512 rows each);
x_j and relations replicated. Each core computes its [8, 512, 4096] slab.

The problem is output-store bound: 64 MiB of fp32 scores per core against
~358 GB/s of HBM bandwidth per core = ~190 us floor. The kernel keeps the
store pipeline saturated and hides matmul (PE) + sigmoid (ACT) under it.

Per-core pipeline:
  - inputs arrive pre-transposed ([D, N] layout, host-side np transpose) so
    the contraction dim D=128 is the SBUF partition dim for both matmul
    operands; no on-device transposes needed.
  - per relation k: scale x_i^T columns by r_k (per-partition tensor_scalar)
  - matmul in bf16 hi/lo 3-pass split (hi*hi + hi*lo + lo*hi, ~1.5e-5
    accurate, 3x faster than native fp32 matmul) or fp32r single pass
  - sigmoid on the scalar engine straight out of PSUM
  - 2 MiB DMA per [128, 4096] result block, alternating between the SP
    hardware DGE ring and the GpSimd software DGE ring
"""

import os

import numpy as np

import concourse.bass as bass
import concourse.mybir as mybir
from concourse import tile
from concourse.bass_utils import run_bass_kernel_spmd

N_I, N_J, D, K = 4096, 4096, 128, 8
N_CORES = 8
SHARD = N_I // N_CORES  # 512
P = 128
HALF = N_J // 2  # 2048
F32 = mybir.dt.float32
F32R = mybir.dt.float32r
BF16 = mybir.dt.bfloat16

# matmul input handling: "split3" = bf16 hi/lo 3-pass (fast, ~3e-5 rel err),
# "f32r" = fp32 round mode (~7e-4 rel err), "fp32" = native fp32 (exact).
MODE = os.environ.get("DISTMULT_MODE", "split3")


def _split_ctrl_waits(nc, maxw=1):
    """walrus in this container accepts only one sync-wait on several
    instruction structs (Drain/TPB_CTRL, tensor_scalar/S3D3_TS, ...); move
    excess waits onto same-engine NOPs placed immediately before. Engines
    consume their queues in order, so waiting on A (NOP) then B (inst) is
    equivalent to the inst waiting on both."""
    for f in nc.m.functions:
        for bb in f.blocks:
            newinsts = []
            for i in bb.instructions:
                si = i.sync_info
                if si is not None and len(si.on_wait) > maxw:
                    waits = list(si.on_wait)
                    extra, keep = waits[:-maxw], waits[-maxw:]
                    for idx in range(0, len(extra), maxw):
                        nop = mybir.InstNoOp(name=f"{i.name}-ws{idx}", ins=[], outs=[])
                        nop.engine = i.engine
                        nop.sync_info = mybir.SyncInfo(
                            on_wait=extra[idx : idx + maxw], on_update=[]
                        )
                        nc.register_instruction(nop)
                        newinsts.append(nop)
                    si.on_wait = keep
                newinsts.append(i)
            bb.instructions[:] = newinsts


def build(mode=MODE):
    nc = bass.Bass()
    x_iT = nc.dram_tensor("x_iT", [D, SHARD], F32, kind="ExternalInput")
    relT = nc.dram_tensor("relT", [D, K], F32, kind="ExternalInput")
    if mode == "split3":
        # duplicated first row-block of x_i^T: a 64 KB load that unblocks the
        # first matmuls ~2us before the full 256 KB x_iT load completes
        x_i0T = nc.dram_tensor("x_i0T", [D, P], F32, kind="ExternalInput")
    if mode == "split3":
        x_jT_hi = nc.dram_tensor("x_jT_hi", [D, N_J], BF16, kind="ExternalInput")
        x_jT_lo = nc.dram_tensor("x_jT_lo", [D, N_J], BF16, kind="ExternalInput")
    else:
        x_jT = nc.dram_tensor("x_jT", [D, N_J], F32R if mode == "f32r" else F32,
                              kind="ExternalInput")
    out = nc.dram_tensor("out", [K, SHARD, N_J], F32, kind="ExternalOutput")

    with tile.TileContext(nc) as tc:
        with (
            tc.tile_pool(name="const", bufs=1) as const,
            tc.tile_pool(name="w", bufs=2) as wpool,
            tc.tile_pool(name="psum", bufs=2, space=bass.MemorySpace.PSUM) as psum,
            tc.tile_pool(name="ob", bufs=4) as obuf,
            tc.tile_pool(name="obs", bufs=6) as obuf_small,
        ):
            if mode == "split3":
                # tiny duplicated loads of the first 512 rhs columns, first in
                # each ring's FIFO, so the leading 512-wide store sub-chunk
                # isn't gated on the full 256 KB rhs chunks
                xjh0a = const.tile([P, 512], BF16, tag="xjh0a")
                nc.sync.dma_start(xjh0a[:], x_jT_hi[:, 0:512])
                xjl0a = const.tile([P, 512], BF16, tag="xjl0a")
                nc.scalar.dma_start(xjl0a[:], x_jT_lo[:, 0:512])
                xi0 = const.tile([P, P], F32, tag="xi0")
                nc.sync.dma_start(xi0[:], x_i0T[:])
            rel = const.tile([P, K], F32, tag="rel")
            nc.sync.dma_start(rel[:], relT[:])
            xiT = const.tile([P, SHARD], F32, tag="xiT")
            nc.scalar.dma_start(xiT[:], x_iT[:])

            # rhs chunks per 2048-wide half; loads alternate HWDGE rings so
            # the first half lands as early as possible.
            if mode == "split3":
                rh, rl = [], []
                for s in range(4):
                    t = const.tile([P, 1024], BF16, tag=f"xjh{s}")
                    nc.sync.dma_start(t[:], x_jT_hi[:, s * 1024 : (s + 1) * 1024])
                    rh.append(t)
                    t = const.tile([P, 1024], BF16, tag=f"xjl{s}")
                    nc.scalar.dma_start(t[:], x_jT_lo[:, s * 1024 : (s + 1) * 1024])
                    rl.append(t)
            else:
                dt = F32R if mode == "f32r" else F32
                rj = []
                for h in range(2):
                    t = const.tile([P, HALF], dt, tag=f"xj{h}")
                    eng = nc.sync if h == 0 else nc.scalar
                    eng.dma_start(t[:], x_jT[:, h * HALF : (h + 1) * HALF])
                    rj.append(t)


            # warm up the sigmoid spline tables (~2.7us) under the input DMAs
            scratch = const.tile([P, 1], F32, tag="scratch")
            nc.gpsimd.memset(scratch[:], 0.0)
            nc.scalar.activation(
                scratch[:], scratch[:], mybir.ActivationFunctionType.Sigmoid
            )

            # warm up the PE clock (HAM un-throttles after ~3.4us of sustained
            # matmul activity) with dummy matmuls while the inputs stream in;
            # otherwise the first ~30us of real matmuls run at 1.2 GHz and
            # the store pipeline ramps slowly.
            wmup = const.tile([P, 512], BF16, tag="wmup")
            nc.gpsimd.memset(wmup[:], 0.0)
            wps = psum.tile([P, HALF], F32, tag="ps")
            for r in range(10):
                nc.tensor.matmul(
                    wps[:, (r % 4) * 512 : (r % 4 + 1) * 512],
                    wmup[:, 0:P],
                    wmup[:],
                    start=True,
                    stop=True,
                )
            # reader keeps the warmup matmuls live through any dead-code pass
            nc.scalar.activation(
                scratch[:], wps[:, 0:1], mybir.ActivationFunctionType.Sigmoid
            )

            if mode == "split3":
                # fast-path k=0 weights for the first 128-row block only:
                # three short DVE ops instead of the full 512-wide chain, so
                # the first matmul triplet is ready ~2us earlier
                wk0 = const.tile([P, P], F32, tag="wk0")
                nc.vector.tensor_scalar_mul(wk0[:], xi0[:], rel[:, 0:1])
                wk0_hi = const.tile([P, P], BF16, tag="wk0_hi")
                nc.vector.tensor_copy(wk0_hi[:], wk0[:])
                wk0_lo = const.tile([P, P], BF16, tag="wk0_lo")
                nc.vector.tensor_sub(wk0_lo[:], wk0[:], wk0_hi[:])

            chunk = 0
            for k in range(K):
                if mode == "split3":
                    wk = wpool.tile([P, SHARD], F32, tag="wk")
                    nc.vector.tensor_scalar_mul(wk[:], xiT[:], rel[:, k : k + 1])
                    wk_hi = wpool.tile([P, SHARD], BF16, tag="wk_hi")
                    nc.vector.tensor_copy(wk_hi[:], wk[:])
                    wk_lo = wpool.tile([P, SHARD], BF16, tag="wk_lo")
                    nc.vector.tensor_sub(wk_lo[:], wk[:], wk_hi[:])
                elif mode == "f32r":
                    wk = wpool.tile([P, SHARD], F32R, tag="wk")
                    nc.vector.tensor_scalar_mul(wk[:], xiT[:], rel[:, k : k + 1])
                else:
                    wk = wpool.tile([P, SHARD], F32, tag="wk")
                    nc.vector.tensor_scalar_mul(wk[:], xiT[:], rel[:, k : k + 1])

                for m in range(SHARD // P):  # 4 row blocks of 128
                    mc = slice(m * P, (m + 1) * P)
                    if mode == "split3" and k == 0 and m == 0:
                        # extra-fine first block: a leading 512-wide sub-chunk
                        # fed from the tiny duplicated loads, then 0.25/0.5 MiB
                        # sub-chunks, so the store stream starts while the PE
                        # is still ramping
                        subs = [
                            (0, 512, xjh0a, xjl0a, 0),
                            (512, 512, rh[0], rl[0], 512),
                            (1024, 1024, rh[1], rl[1], 0),
                            (2048, 1024, rh[2], rl[2], 0),
                            (3072, 1024, rh[3], rl[3], 0),
                        ]
                        for c0, w, th, tl, off in subs:
                            psq = psum.tile([P, w], F32, tag="ps")
                            for n2 in range(w // 512):
                                psl = psq[:, n2 * 512 : (n2 + 1) * 512]
                                rsl = slice(off + n2 * 512, off + (n2 + 1) * 512)
                                nc.tensor.matmul(
                                    psl, wk0_hi[:], th[:, rsl],
                                    start=True, stop=False,
                                )
                                nc.tensor.matmul(
                                    psl, wk0_hi[:], tl[:, rsl],
                                    start=False, stop=False,
                                )
                                nc.tensor.matmul(
                                    psl, wk0_lo[:], th[:, rsl],
                                    start=False, stop=True,
                                )
                            obq = obuf_small.tile([P, w], F32, tag="obs")
                            nc.scalar.activation(
                                obq[:], psq[:], mybir.ActivationFunctionType.Sigmoid
                            )
                            eng = nc.sync if chunk % 2 == 0 else nc.gpsimd
                            eng.dma_start(out[0, 0:P, c0 : c0 + w], obq[:])
                            chunk += 1
                        continue
                    # 1 MiB store granularity for the last block (shorter
                    # drain); 2 MiB blocks elsewhere (fewer sems, shorter
                    # kernel-tail sem-clear storm).
                    fine = k == K - 1 and m == SHARD // P - 1
                    ob = None if fine else obuf.tile([P, N_J], F32, tag="ob")
                    for h in range(2):  # two 2048-wide PSUM tiles per block
                        ps = psum.tile([P, HALF], F32, tag="ps")
                        for n4 in range(4):  # one 512-wide matmul per bank
                            cs = slice(n4 * 512, (n4 + 1) * 512)
                            psl = ps[:, cs]
                            if mode == "split3":
                                gc = h * HALF + n4 * 512
                                rsl = slice(gc % 1024, gc % 1024 + 512)
                                w_hi = (wk0_hi[:], wk_hi[:, mc])[0 if (k == 0 and m == 0) else 1]
                                w_lo = (wk0_lo[:], wk_lo[:, mc])[0 if (k == 0 and m == 0) else 1]
                                nc.tensor.matmul(
                                    psl, w_hi, rh[gc // 1024][:, rsl],
                                    start=True, stop=False,
                                )
                                nc.tensor.matmul(
                                    psl, w_hi, rl[gc // 1024][:, rsl],
                                    start=False, stop=False,
                                )
                                nc.tensor.matmul(
                                    psl, w_lo, rh[gc // 1024][:, rsl],
                                    start=False, stop=True,
                                )
                            else:
                                nc.tensor.matmul(
                                    psl, wk[:, mc], rj[h][:, cs],
                                    start=True, stop=True,
                                )
                        if fine:
                            if h == 0:
                                obh = obuf_small.tile([P, HALF], F32, tag="obs")
                                nc.scalar.activation(
                                    obh[:], ps[:],
                                    mybir.ActivationFunctionType.Sigmoid,
                                )
                                nc.sync.dma_start(out[k, mc, 0:HALF], obh[:])
                            else:
                                # taper the very last stores (1024+512+512) so
                                # the kernel-final DMA is only 0.25 MiB of
                                # data + receipt before the drain
                                for o0, w, eng in (
                                    (0, 1024, nc.scalar),
                                    (1024, 512, nc.sync),
                                    (1536, 512, nc.scalar),
                                ):
                                    obt = obuf_small.tile([P, w], F32, tag="obs")
                                    nc.scalar.activation(
                                        obt[:], ps[:, o0 : o0 + w],
                                        mybir.ActivationFunctionType.Sigmoid,
                                    )
                                    eng.dma_start(
                                        out[k, mc, HALF + o0 : HALF + o0 + w],
                                        obt[:],
                                    )
                            chunk += 1
                        else:
                            nc.scalar.activation(
                                ob[:, h * HALF : (h + 1) * HALF],
                                ps[:],
                                mybir.ActivationFunctionType.Sigmoid,
                            )
                    if not fine:
                        eng = nc.sync if chunk % 2 == 0 else nc.gpsimd
                        eng.dma_start(out[k, mc, :], ob[:])
                        chunk += 1

    _split_ctrl_waits(nc)
    return nc


_cache = {}


def kernel(x_i, x_j, relations):
    x_i = np.asarray(x_i, dtype=np.float32)
    x_j = np.asarray(x_j, dtype=np.float32)
    relations = np.asarray(relations, dtype=np.float32)
    assert x_i.shape == (N_I, D) and x_j.shape == (N_J, D)
    assert relations.shape == (K, D)

    if MODE not in _cache:
        _cache[MODE] = build(MODE)
    nc = _cache[MODE]

    x_jT = np.ascontiguousarray(x_j.T)
    relT = np.ascontiguousarray(relations.T)
    common = {"relT": relT}
    if MODE == "split3":
        import ml_dtypes

        hi = x_jT.astype(ml_dtypes.bfloat16)
        lo = (x_jT - hi.astype(np.float32)).astype(ml_dtypes.bfloat16)
        common["x_jT_hi"] = hi
        common["x_jT_lo"] = lo
    else:
        common["x_jT"] = x_jT

    in_maps = []
    for c in range(N_CORES):
        shard = np.ascontiguousarray(x_i[c * SHARD : (c + 1) * SHARD, :].T)
        m = {"x_iT": shard, **common}
        if MODE == "split3":
            m["x_i0T"] = np.ascontiguousarray(shard[:, 0:P])
        in_maps.append(m)

    trace = bool(int(os.environ.get("DISTMULT_TRACE", "0")))
    res = run_bass_kernel_spmd(nc, in_maps, list(range(N_CORES)), trace=trace)
    if trace:
        kernel.last_exec_time_ns = res.exec_time_ns
        kernel.last_results = res
    return np.concatenate([res.results[c]["out"] for c in range(N_CORES)], axis=1)



# revision 2
# speedup vs baseline: 1.3280x; 1.3280x over previous
"""DistMult decoder on 8 Trainium2 NeuronCores.

reference: out[k, i, j] = sigmoid( sum_d x_i[i, d] * relations[k, d] * x_j[j, d] )
shapes: x_i [4096, 128] f32, x_j [4096, 128] f32, relations [8, 128] f32
output: [8, 4096, 4096] f32 (512 MiB)

Sharding: rows of x_i (N_i axis) split across the 8 cores (512 rows each);
x_j and relations replicated. Each core computes its [8, 512, 4096] slab.

The scores are stored as bf16 (sigmoid output is in [0,1]; bf16 quantization
adds <2e-3 abs error against a 2e-2 budget) and widened to f32 on the host.
That halves the HBM store traffic to 32 MiB/core, which moves the bottleneck
from the store stream (~187 us for f32) to the ScalarE sigmoid:
ACTIVATE runs at 1 elem/lane/cycle @ 1.2 GHz with ~370ns/instruction overhead,
so 16.8M sigmoids in [128, 2048] PSUM chunks = 64 * 1.89us = ~121 us.
PE (2-pass bf16 matmul, 110 us) and DMA (32 MiB, ~90 us) hide under it.

Per-core pipeline:
  - inputs arrive pre-transposed ([D, N] layout, host-side np transpose) so
    the contraction dim D=128 is the SBUF partition dim for both matmul
    operands; no on-device transposes needed.
  - per relation k: scale x_i^T columns by r_k (per-partition tensor_scalar),
    split into bf16 hi + lo halves (2-pass matmul: (hi+lo)*xj_hi keeps the
    max abs error at ~8e-3 vs ~1.2e-2 for a single bf16 pass)
  - matmul 512-col chunks into [128, 2048] PSUM tiles (4 banks, 2-deep pool)
  - sigmoid on the scalar engine straight out of PSUM, bf16 into SBUF
  - 1 MiB DMA per [128, 4096] result block, alternating between the SP
    hardware DGE ring and the GpSimd software DGE ring
"""

import os

import numpy as np

import concourse.bass as bass
import concourse.mybir as mybir
from concourse import tile
from concourse.bass_utils import run_bass_kernel_spmd

N_I, N_J, D, K = 4096, 4096, 128, 8
N_CORES = 8
SHARD = N_I // N_CORES  # 512
P = 128
HALF = N_J // 2  # 2048
F32 = mybir.dt.float32
BF16 = mybir.dt.bfloat16

# matmul passes: "bf2" = bf16 hi/lo 2-pass weights (~8e-3 max err),
# "bf1" = single bf16 pass (~1.2e-2 max err, half the PE time).
MODE = os.environ.get("DISTMULT_MODE", "bf2")


def _split_ctrl_waits(nc, maxw=1):
    """walrus in this container accepts only one sync-wait on several
    instruction structs (Drain/TPB_CTRL, tensor_scalar/S3D3_TS, ...); move
    excess waits onto same-engine NOPs placed immediately before. Engines
    consume their queues in order, so waiting on A (NOP) then B (inst) is
    equivalent to the inst waiting on both."""
    for f in nc.m.functions:
        for bb in f.blocks:
            newinsts = []
            for i in bb.instructions:
                si = i.sync_info
                if si is not None and len(si.on_wait) > maxw:
                    waits = list(si.on_wait)
                    extra, keep = waits[:-maxw], waits[-maxw:]
                    for idx in range(0, len(extra), maxw):
                        nop = mybir.InstNoOp(name=f"{i.name}-ws{idx}", ins=[], outs=[])
                        nop.engine = i.engine
                        nop.sync_info = mybir.SyncInfo(
                            on_wait=extra[idx : idx + maxw], on_update=[]
                        )
                        nc.register_instruction(nop)
                        newinsts.append(nop)
                    si.on_wait = keep
                newinsts.append(i)
            bb.instructions[:] = newinsts


def build(mode=MODE):
    nc = bass.Bass()
    x_iT = nc.dram_tensor("x_iT", [D, SHARD], F32, kind="ExternalInput")
    # duplicated first row-block of x_i^T: a 64 KB load that unblocks the
    # first matmuls before the full 256 KB x_iT load completes
    x_i0T = nc.dram_tensor("x_i0T", [D, P], F32, kind="ExternalInput")
    relT = nc.dram_tensor("relT", [D, K], F32, kind="ExternalInput")
    x_jT = nc.dram_tensor("x_jT", [D, N_J], BF16, kind="ExternalInput")
    out = nc.dram_tensor("out", [K, SHARD, N_J], BF16, kind="ExternalOutput")

    with tile.TileContext(nc) as tc:
        with (
            tc.tile_pool(name="const", bufs=1) as const,
            tc.tile_pool(name="w", bufs=2) as wpool,
            tc.tile_pool(name="psum", bufs=2, space=bass.MemorySpace.PSUM) as psum,
            tc.tile_pool(name="ob", bufs=4) as obuf,
            tc.tile_pool(name="obs", bufs=6) as obuf_small,
        ):
            # rhs chunks, small-first and alternating HWDGE rings so the
            # leading columns land as early as possible
            xj_chunks = []  # (col0, width, tile)
            for i, (c0, w) in enumerate(
                [(0, 512), (512, 512), (1024, 1024), (2048, 1024), (3072, 1024)]
            ):
                t = const.tile([P, w], BF16, tag=f"xj{i}")
                eng = nc.sync if i % 2 == 0 else nc.scalar
                eng.dma_start(t[:], x_jT[:, c0 : c0 + w])
                xj_chunks.append((c0, w, t))

            def rhs(col0, width):
                """tile slice covering global cols [col0, col0+width)"""
                for c0, w, t in xj_chunks:
                    if c0 <= col0 and col0 + width <= c0 + w:
                        return t[:, col0 - c0 : col0 - c0 + width]
                raise AssertionError((col0, width))

            xi0 = const.tile([P, P], F32, tag="xi0")
            nc.sync.dma_start(xi0[:], x_i0T[:])
            rel = const.tile([P, K], F32, tag="rel")
            nc.sync.dma_start(rel[:], relT[:])
            xiT = const.tile([P, SHARD], F32, tag="xiT")
            nc.scalar.dma_start(xiT[:], x_iT[:])

            # warm up the sigmoid spline tables (~2.7us) under the input DMAs
            scratch = const.tile([P, 1], F32, tag="scratch")
            nc.gpsimd.memset(scratch[:], 0.0)
            nc.scalar.activation(
                scratch[:], scratch[:], mybir.ActivationFunctionType.Sigmoid
            )

            # warm up the PE clock (HAM un-throttles after ~3.4us of sustained
            # matmul activity) with dummy matmuls while the inputs stream in;
            # otherwise the first ~30us of real matmuls run at 1.2 GHz.
            wmup = const.tile([P, 512], BF16, tag="wmup")
            nc.gpsimd.memset(wmup[:], 0.0)
            wps = psum.tile([P, HALF], F32, tag="ps")
            for r in range(10):
                nc.tensor.matmul(
                    wps[:, (r % 4) * 512 : (r % 4 + 1) * 512],
                    wmup[:, 0:P],
                    wmup[:],
                    start=True,
                    stop=True,
                )
            # reader keeps the warmup matmuls live through any dead-code pass
            nc.scalar.activation(
                scratch[:], wps[:, 0:1], mybir.ActivationFunctionType.Sigmoid
            )

            # fast-path k=0 weights for the first 128-row block only: three
            # short DVE ops instead of the full 512-wide chain, so the first
            # matmuls are ready earlier
            wk0 = const.tile([P, P], F32, tag="wk0")
            nc.vector.tensor_scalar_mul(wk0[:], xi0[:], rel[:, 0:1])
            wk0_hi = const.tile([P, P], BF16, tag="wk0_hi")
            nc.vector.tensor_copy(wk0_hi[:], wk0[:])
            if mode == "bf2":
                wk0_lo = const.tile([P, P], BF16, tag="wk0_lo")
                nc.vector.tensor_sub(wk0_lo[:], wk0[:], wk0_hi[:])

            chunk = 0
            for k in range(K):
                wk = wpool.tile([P, SHARD], F32, tag="wk")
                nc.vector.tensor_scalar_mul(wk[:], xiT[:], rel[:, k : k + 1])
                wk_hi = wpool.tile([P, SHARD], BF16, tag="wk_hi")
                nc.vector.tensor_copy(wk_hi[:], wk[:])
                if mode == "bf2":
                    wk_lo = wpool.tile([P, SHARD], BF16, tag="wk_lo")
                    nc.vector.tensor_sub(wk_lo[:], wk[:], wk_hi[:])

                for m in range(SHARD // P):  # 4 row blocks of 128
                    mc = slice(m * P, (m + 1) * P)
                    first = k == 0 and m == 0
                    lhs_hi = wk0_hi[:] if first else wk_hi[:, mc]
                    if mode == "bf2":
                        lhs_lo = wk0_lo[:] if first else wk_lo[:, mc]
                    # 1 MiB store granularity; taper the very last block so
                    # the kernel-final DMA is small (short drain).
                    fine = k == K - 1 and m == SHARD // P - 1
                    ob = None if fine else obuf.tile([P, N_J], BF16, tag="ob")
                    for h in range(2):  # two 2048-wide PSUM tiles per block
                        ps = psum.tile([P, HALF], F32, tag="ps")
                        for n4 in range(4):  # one 512-wide matmul per bank
                            psl = ps[:, n4 * 512 : (n4 + 1) * 512]
                            r512 = rhs(h * HALF + n4 * 512, 512)
                            if mode == "bf2":
                                nc.tensor.matmul(
                                    psl, lhs_hi, r512, start=True, stop=False
                                )
                                nc.tensor.matmul(
                                    psl, lhs_lo, r512, start=False, stop=True
                                )
                            else:
                                nc.tensor.matmul(
                                    psl, lhs_hi, r512, start=True, stop=True
                                )
                        if fine:
                            obh = obuf_small.tile([P, HALF], BF16, tag="obs")
                            nc.scalar.activation(
                                obh[:], ps[:], mybir.ActivationFunctionType.Sigmoid
                            )
                            if h == 0:
                                nc.sync.dma_start(out[k, mc, 0:HALF], obh[:])
                            else:
                                for o0, w, eng in (
                                    (0, 1024, nc.scalar),
                                    (1024, 512, nc.sync),
                                    (1536, 512, nc.scalar),
                                ):
                                    eng.dma_start(
                                        out[k, mc, HALF + o0 : HALF + o0 + w],
                                        obh[:, o0 : o0 + w],
                                    )
                        else:
                            nc.scalar.activation(
                                ob[:, h * HALF : (h + 1) * HALF],
                                ps[:],
                                mybir.ActivationFunctionType.Sigmoid,
                            )
                    if not fine:
                        eng = nc.sync if chunk % 2 == 0 else nc.gpsimd
                        eng.dma_start(out[k, mc, :], ob[:])
                        chunk += 1

    _split_ctrl_waits(nc)
    return nc


_cache = {}


def kernel(x_i, x_j, relations):
    import ml_dtypes

    x_i = np.asarray(x_i, dtype=np.float32)
    x_j = np.asarray(x_j, dtype=np.float32)
    relations = np.asarray(relations, dtype=np.float32)
    assert x_i.shape == (N_I, D) and x_j.shape == (N_J, D)
    assert relations.shape == (K, D)

    if MODE not in _cache:
        _cache[MODE] = build(MODE)
    nc = _cache[MODE]

    x_jT = np.ascontiguousarray(x_j.T).astype(ml_dtypes.bfloat16)
    relT = np.ascontiguousarray(relations.T)
    common = {"relT": relT, "x_jT": x_jT}

    in_maps = []
    for c in range(N_CORES):
        shard = np.ascontiguousarray(x_i[c * SHARD : (c + 1) * SHARD, :].T)
        m = {"x_iT": shard, "x_i0T": np.ascontiguousarray(shard[:, 0:P]), **common}
        in_maps.append(m)

    trace = bool(int(os.environ.get("DISTMULT_TRACE", "0")))
    res = run_bass_kernel_spmd(nc, in_maps, list(range(N_CORES)), trace=trace)
    if trace:
        kernel.last_exec_time_ns = res.exec_time_ns
        kernel.last_results = res
    return np.concatenate(
        [res.results[c]["out"].astype(np.float32) for c in range(N_CORES)], axis=1
    )


# revision 3
# speedup vs baseline: 1.4181x; 1.0679x over previous
"""DistMult decoder on 8 Trainium2 NeuronCores.

reference: out[k, i, j] = sigmoid( sum_d x_i[i, d] * relations[k, d] * x_j[j, d] )
shapes: x_i [4096, 128] f32, x_j [4096, 128] f32, relations [8, 128] f32
output: [8, 4096, 4096] f32 (512 MiB)

Sharding: rows of x_i (N_i axis) split across the 8 cores (512 rows each);
x_j and relations replicated. Each core computes its [8, 512, 4096] slab.

The scores are stored as bf16 (sigmoid output is in [0,1]; bf16 quantization
adds <2e-3 abs error against a 2e-2 budget) and widened to f32 on the host.
That halves the HBM store traffic to 32 MiB/core, which moves the bottleneck
from the store stream (~187 us for f32) to the ScalarE sigmoid:
ACTIVATE runs at 1 elem/lane/cycle @ 1.2 GHz with ~370 ns/instruction access
overhead, so 16.8M sigmoids in [128, 2048] PSUM chunks = 64 * ~1.87us + sem
waits = ~128 us of ScalarE time. Everything else hides under it:
PE (single-pass bf16 matmul, ~62 us), DMA (32 MiB out, ~90 us), DVE (~4 us).

Per-core pipeline:
  - inputs arrive pre-transposed ([D, N] layout, host-side np transpose) so
    the contraction dim D=128 is the SBUF partition dim for both matmul
    operands; no on-device transposes needed.
  - per relation k: weights = bf16(x_i^T * r_k) via one per-partition
    tensor_scalar on DVE ("bf1" mode, ~1.2e-2 max err; "bf2" splits the
    weights hi/lo for a 2-pass matmul, ~8e-3 max err at 2x the PE time)
  - matmul 512-col chunks into [128, 2048] PSUM tiles (4 banks, 2-deep pool)
  - sigmoid on the scalar engine straight out of PSUM, bf16 into SBUF;
    nothing else runs on ScalarE (no DMA dispatch, no copies)
  - 1 MiB DMA per [128, 4096] result block, alternating between the SP
    hardware DGE ring and the GpSimd software DGE ring
  - first output block is computed in 512/1024-wide sub-chunks fed by
    small leading input DMAs so ScalarE starts ~1.5 us earlier; the last
    block's stores taper down to 128 KiB so the final drain is short
"""

import os

import numpy as np

import concourse.bass as bass
import concourse.mybir as mybir
from concourse import tile
from concourse.bass_utils import run_bass_kernel_spmd

N_I, N_J, D, K = 4096, 4096, 128, 8
N_CORES = 8
SHARD = N_I // N_CORES  # 512
P = 128
HALF = N_J // 2  # 2048
F32 = mybir.dt.float32
BF16 = mybir.dt.bfloat16
SIG = mybir.ActivationFunctionType.Sigmoid

# matmul passes: "bf1" = single bf16 pass (~1.2e-2 max err),
# "bf2" = bf16 hi/lo 2-pass weights (~8e-3 max err, 2x PE time).
MODE = os.environ.get("DISTMULT_MODE", "bf1")


def _split_ctrl_waits(nc, maxw=1):
    """walrus in this container accepts only one sync-wait on several
    instruction structs (Drain/TPB_CTRL, tensor_scalar/S3D3_TS, ...); move
    excess waits onto same-engine NOPs placed immediately before. Engines
    consume their queues in order, so waiting on A (NOP) then B (inst) is
    equivalent to the inst waiting on both."""
    for f in nc.m.functions:
        for bb in f.blocks:
            newinsts = []
            for i in bb.instructions:
                si = i.sync_info
                if si is not None and len(si.on_wait) > maxw:
                    waits = list(si.on_wait)
                    extra, keep = waits[:-maxw], waits[-maxw:]
                    for idx in range(0, len(extra), maxw):
                        nop = mybir.InstNoOp(name=f"{i.name}-ws{idx}", ins=[], outs=[])
                        nop.engine = i.engine
                        nop.sync_info = mybir.SyncInfo(
                            on_wait=extra[idx : idx + maxw], on_update=[]
                        )
                        nc.register_instruction(nop)
                        newinsts.append(nop)
                    si.on_wait = keep
                newinsts.append(i)
            bb.instructions[:] = newinsts


def build(mode=MODE):
    nc = bass.Bass()
    x_iT = nc.dram_tensor("x_iT", [D, SHARD], F32, kind="ExternalInput")
    # duplicated first row-block of x_i^T: a 64 KB load that unblocks the
    # first matmuls before the full 256 KB x_iT load completes
    x_i0T = nc.dram_tensor("x_i0T", [D, P], F32, kind="ExternalInput")
    relT = nc.dram_tensor("relT", [D, K], F32, kind="ExternalInput")
    x_jT = nc.dram_tensor("x_jT", [D, N_J], BF16, kind="ExternalInput")
    out = nc.dram_tensor("out", [K, SHARD, N_J], BF16, kind="ExternalOutput")

    with tile.TileContext(nc) as tc:
        with (
            tc.tile_pool(name="const", bufs=1) as const,
            tc.tile_pool(name="w", bufs=2) as wpool,
            tc.tile_pool(name="psum", bufs=2, space=bass.MemorySpace.PSUM) as psum,
            tc.tile_pool(name="ob", bufs=4) as obuf,
            tc.tile_pool(name="obs", bufs=6) as obuf_small,
        ):
            # small unblock loads first: weights chain, then leading rhs cols
            xi0 = const.tile([P, P], F32, tag="xi0")
            nc.sync.dma_start(xi0[:], x_i0T[:])
            rel = const.tile([P, K], F32, tag="rel")
            nc.sync.dma_start(rel[:], relT[:])

            # rhs chunks; leading columns small and early, bulk on both rings
            xj_chunks = []  # (col0, width, tile)
            for i, (c0, w, eng) in enumerate(
                [
                    (0, 512, nc.sync),
                    (512, 512, nc.gpsimd),
                    (1024, 1024, nc.sync),
                    (2048, 1024, nc.gpsimd),
                    (3072, 1024, nc.sync),
                ]
            ):
                t = const.tile([P, w], BF16, tag=f"xj{i}")
                eng.dma_start(t[:], x_jT[:, c0 : c0 + w])
                xj_chunks.append((c0, w, t))

            def rhs(col0, width):
                """tile slice covering global cols [col0, col0+width)"""
                for c0, w, t in xj_chunks:
                    if c0 <= col0 and col0 + width <= c0 + w:
                        return t[:, col0 - c0 : col0 - c0 + width]
                raise AssertionError((col0, width))

            xiT = const.tile([P, SHARD], F32, tag="xiT")
            nc.sync.dma_start(xiT[:], x_iT[:])

            # warm up the sigmoid spline tables (~2.7us) under the input DMAs
            scratch = const.tile([P, 1], F32, tag="scratch")
            nc.vector.memset(scratch[:], 0.0)
            nc.scalar.activation(scratch[:], scratch[:], SIG)

            # nudge the PE clock (HAM un-throttles after ~3.4us of sustained
            # matmul activity) with a few dummy matmuls while the inputs
            # stream in; the real matmul stream then finishes the ramp.
            wmup = const.tile([P, 512], BF16, tag="wmup")
            nc.vector.memset(wmup[:], 0.0)
            wps = psum.tile([P, HALF], F32, tag="ps")
            for r in range(4):
                nc.tensor.matmul(
                    wps[:, r * 512 : (r + 1) * 512],
                    wmup[:, 0:P],
                    wmup[:],
                    start=True,
                    stop=True,
                )
            # reader keeps the warmup matmuls live through any dead-code pass
            nc.scalar.activation(scratch[:], wps[:, 0:1], SIG)

            # fast-path k=0 weights for the first 128-row block only, so the
            # first matmuls are ready as early as possible
            wk0_hi = const.tile([P, P], BF16, tag="wk0_hi")
            if mode == "bf2":
                wk0 = const.tile([P, P], F32, tag="wk0")
                nc.vector.tensor_scalar_mul(wk0[:], xi0[:], rel[:, 0:1])
                nc.vector.tensor_copy(wk0_hi[:], wk0[:])
                wk0_lo = const.tile([P, P], BF16, tag="wk0_lo")
                nc.vector.tensor_sub(wk0_lo[:], wk0[:], wk0_hi[:])
            else:
                nc.vector.tensor_scalar_mul(wk0_hi[:], xi0[:], rel[:, 0:1])

            def matmuls(ps_slice, lhs_hi, lhs_lo, col0, width):
                """fill a PSUM slice from 512-col matmul chunks; stationary
                weights grouped (all hi passes, then all lo) so LDWEIGHTS is
                amortized across 4 matmuls instead of reloading per-chunk."""
                nbank = width // 512
                for n4 in range(nbank):
                    nc.tensor.matmul(
                        ps_slice[:, n4 * 512 : (n4 + 1) * 512],
                        lhs_hi,
                        rhs(col0 + n4 * 512, 512),
                        start=True,
                        stop=lhs_lo is None,
                    )
                if lhs_lo is not None:
                    for n4 in range(nbank):
                        nc.tensor.matmul(
                            ps_slice[:, n4 * 512 : (n4 + 1) * 512],
                            lhs_lo,
                            rhs(col0 + n4 * 512, 512),
                            start=False,
                            stop=True,
                        )

            chunk = 0
            for k in range(K):
                if mode == "bf2":
                    wk = wpool.tile([P, SHARD], F32, tag="wk")
                    nc.vector.tensor_scalar_mul(wk[:], xiT[:], rel[:, k : k + 1])
                    wk_hi = wpool.tile([P, SHARD], BF16, tag="wk_hi")
                    nc.vector.tensor_copy(wk_hi[:], wk[:])
                    wk_lo = wpool.tile([P, SHARD], BF16, tag="wk_lo")
                    nc.vector.tensor_sub(wk_lo[:], wk[:], wk_hi[:])
                else:
                    wk_hi = wpool.tile([P, SHARD], BF16, tag="wk_hi")
                    nc.vector.tensor_scalar_mul(wk_hi[:], xiT[:], rel[:, k : k + 1])

                for m in range(SHARD // P):  # 4 row blocks of 128
                    mc = slice(m * P, (m + 1) * P)
                    first = k == 0 and m == 0
                    lhs_hi = wk0_hi[:] if first else wk_hi[:, mc]
                    lhs_lo = None
                    if mode == "bf2":
                        lhs_lo = wk0_lo[:] if first else wk_lo[:, mc]
                    fine = k == K - 1 and m == SHARD // P - 1
                    ob = None if fine else obuf.tile([P, N_J], BF16, tag="ob")
                    for h in range(2):  # two 2048-wide PSUM tiles per block
                        ps = psum.tile([P, HALF], F32, tag="ps")
                        c0 = h * HALF
                        if first and h == 0:
                            # extra-fine first tile: sigmoid in 512/512/1024
                            # sub-chunks so ScalarE starts as soon as the
                            # first 512-col matmul lands
                            for s0, w in ((0, 512), (512, 512), (1024, 1024)):
                                matmuls(ps[:, s0 : s0 + w], lhs_hi, lhs_lo, s0, w)
                                nc.scalar.activation(
                                    ob[:, s0 : s0 + w], ps[:, s0 : s0 + w], SIG
                                )
                            continue
                        matmuls(ps[:], lhs_hi, lhs_lo, c0, HALF)
                        if fine:
                            if h == 0:
                                obh = obuf_small.tile([P, HALF], BF16, tag="obs")
                                nc.scalar.activation(obh[:], ps[:], SIG)
                                nc.sync.dma_start(out[k, mc, 0:HALF], obh[:])
                            else:
                                # taper: 2 sigmoid halves, stores shrinking to
                                # 128 KiB so the kernel-final DMA drains fast
                                for s0, w in ((0, 1024), (1024, 1024)):
                                    obt = obuf_small.tile([P, w], BF16, tag="obs")
                                    nc.scalar.activation(
                                        obt[:], ps[:, s0 : s0 + w], SIG
                                    )
                                    if s0 == 0:
                                        nc.gpsimd.dma_start(
                                            out[k, mc, HALF : HALF + 1024], obt[:]
                                        )
                                    else:
                                        nc.sync.dma_start(
                                            out[k, mc, HALF + 1024 : HALF + 1536],
                                            obt[:, 0:512],
                                        )
                                        nc.gpsimd.dma_start(
                                            out[k, mc, HALF + 1536 : N_J],
                                            obt[:, 512:1024],
                                        )
                        else:
                            nc.scalar.activation(
                                ob[:, c0 : c0 + HALF], ps[:], SIG
                            )
                    if not fine:
                        eng = nc.sync if chunk % 2 == 0 else nc.gpsimd
                        eng.dma_start(out[k, mc, :], ob[:])
                        chunk += 1

    _split_ctrl_waits(nc)
    return nc


_cache = {}


def kernel(x_i, x_j, relations):
    import ml_dtypes

    x_i = np.asarray(x_i, dtype=np.float32)
    x_j = np.asarray(x_j, dtype=np.float32)
    relations = np.asarray(relations, dtype=np.float32)
    assert x_i.shape == (N_I, D) and x_j.shape == (N_J, D)
    assert relations.shape == (K, D)

    if MODE not in _cache:
        _cache[MODE] = build(MODE)
    nc = _cache[MODE]

    x_jT = np.ascontiguousarray(x_j.T).astype(ml_dtypes.bfloat16)
    relT = np.ascontiguousarray(relations.T)
    common = {"relT": relT, "x_jT": x_jT}

    in_maps = []
    for c in range(N_CORES):
        shard = np.ascontiguousarray(x_i[c * SHARD : (c + 1) * SHARD, :].T)
        m = {"x_iT": shard, "x_i0T": np.ascontiguousarray(shard[:, 0:P]), **common}
        in_maps.append(m)

    trace = bool(int(os.environ.get("DISTMULT_TRACE", "0")))
    res = run_bass_kernel_spmd(nc, in_maps, list(range(N_CORES)), trace=trace)
    if trace:
        kernel.last_exec_time_ns = res.exec_time_ns
        kernel.last_results = res
    return np.concatenate(
        [res.results[c]["out"].astype(np.float32) for c in range(N_CORES)], axis=1
    )
